# revision 21
# baseline (speedup 1.0000x reference)
import sys

import numpy as np

try:
    import concourse.bass as bass
except ImportError:
    sys.path.insert(0, "/opt/trn_rl_repo")
    import concourse.bass as bass

import concourse.bacc as bacc
import concourse.mybir as mybir
import concourse.tile as tile
from concourse.bass_utils import run_bass_kernel_spmd

import ml_dtypes

F32 = mybir.dt.float32
BF16 = mybir.dt.bfloat16
NPBF16 = ml_dtypes.bfloat16
B, S, D = 2, 2048, 1024
NH, DH = 16, 64
HPC = 4            # heads per core
HF = HPC * DH      # 256 per-core head features
TQ = S // 4        # 512: t-chunk / i-chunk width
NJT = S // 128     # 16 j-tiles of 128
SCALE = 1.0 / float(np.sqrt(DH))

_CACHE = {}


def _build_graph(variant="full", reps=1):
    nc = bacc.Bacc(num_devices=8)

    xqT = nc.dram_tensor("xqT", [D, S], BF16, kind="ExternalInput")
    xkT = nc.dram_tensor("xkT", [D, S], BF16, kind="ExternalInput")
    xvT = nc.dram_tensor("xvT", [D, S], BF16, kind="ExternalInput")
    wqT = nc.dram_tensor("wqT", [D, HF], BF16, kind="ExternalInput")
    wkT = nc.dram_tensor("wkT", [D, HF], BF16, kind="ExternalInput")
    wvT = nc.dram_tensor("wvT", [D, HF], BF16, kind="ExternalInput")
    woT = nc.dram_tensor("woT", [D, D], BF16, kind="ExternalInput")
    dmask = nc.dram_tensor("dmask", [128, 128], F32, kind="ExternalInput")
    out_q = nc.dram_tensor("out_q", [TQ, D], F32, kind="ExternalOutput")

    GROUPS = [[0, 1, 2, 3], [4, 5, 6, 7]]
    Exp = mybir.ActivationFunctionType.Exp

    with tile.TileContext(nc) as tc:
        # This core's rank within its 4-core gather group: selects which
        # 128-position strip of each chunk's output this core computes.
        rank = nc.sync.snap(
            nc.sync.cc_rank(replica_groups=GROUPS), donate=True,
            min_val=0, max_val=3,
        )
        with (
            tc.tile_pool(name="dram", bufs=1, space="DRAM") as dramp,
            tc.tile_pool(name="const", bufs=1) as constp,
            tc.tile_pool(name="persist", bufs=1) as pers,
            tc.tile_pool(name="weights", bufs=1) as wpool,
            tc.tile_pool(name="xstage", bufs=3) as xpool,
            tc.tile_pool(name="attn", bufs=4) as apool,
            tc.tile_pool(name="ctx", bufs=2) as cpool,
            tc.tile_pool(name="rb", bufs=2) as rbpool,
            tc.tile_pool(name="rv", bufs=2) as rvpool,
            tc.tile_pool(name="obuf", bufs=3) as obp,
            tc.tile_pool(name="cstage", bufs=2) as cstp,
            tc.tile_pool(name="ps_mm", bufs=2, space="PSUM") as ps_mm,
            tc.tile_pool(name="ps_s", bufs=3, space="PSUM") as ps_s,
            tc.tile_pool(name="ps_ctx", bufs=2, space="PSUM") as ps_ctx,
            tc.tile_pool(name="ps_b", bufs=1, space="PSUM") as ps_b,
        ):
            ccin = [dramp.tile([HF, TQ], BF16, name=f"ccin{j}") for j in range(4)]
            agout = [
                dramp.tile([4 * HF, TQ], BF16, name=f"agout{j}") for j in range(4)
            ]

            dmask_sb = constp.tile([128, 128], F32, name="dmask_sb")
            nc.sync.dma_start(dmask_sb[:], dmask[:, :])
            ones_sb = constp.tile([1, DH], BF16, name="ones_sb")
            nc.vector.memset(ones_sb[:], 1.0)

            wq_sb = wpool.tile([128, 8, HF], BF16, name="wq_sb")
            wk_sb = wpool.tile([128, 8, HF], BF16, name="wk_sb")
            wv_sb = wpool.tile([128, 8, HF], BF16, name="wv_sb")
            wo_sb = wpool.tile([128, 8, D], BF16, name="wo_sb")

            QT = [pers.tile([128, S], BF16, name=f"QT{u}") for u in range(2)]
            KT = [pers.tile([128, S], BF16, name=f"KT{u}") for u in range(2)]
            Vb = pers.tile([128, NJT * HPC, DH + 1], BF16, name="Vb")
            nc.vector.memset(Vb[:, :, DH], 1.0)

            def proj_units(tcc, first):
                """Generator: x-chunk DMAs + Q/K/V projections for t-chunk
                tcc, yielding after each small unit of work so it can be
                interleaved into the previous chunk's (ACT-gated) attention
                loop, keeping the in-order PE queue mixed ~1:1."""
                xq_sb = xpool.tile([128, 8, TQ], BF16, name="xst")
                xk_sb = xpool.tile([128, 8, TQ], BF16, name="xst")
                xv_sb = xpool.tile([128, 8, TQ], BF16, name="xst")
                srcs = ((xq_sb, xqT), (xk_sb, xkT), (xv_sb, xvT))
                for i, (xs, src) in enumerate(srcs):
                    nc.sync.dma_start(
                        xs[:],
                        src[:, bass.ts(tcc, TQ)].rearrange(
                            "(n p) t -> p n t", p=128
                        ),
                    )
                    if first and i == 0:
                        # Weights for the projections; wo much later (it is
                        # only needed by the first outproj, ~40us in).
                        nc.sync.dma_start(
                            wq_sb[:],
                            wqT[:, :].rearrange("(n p) o -> p n o", p=128))
                        nc.sync.dma_start(
                            wk_sb[:],
                            wkT[:, :].rearrange("(n p) o -> p n o", p=128))
                        nc.sync.dma_start(
                            wv_sb[:],
                            wvT[:, :].rearrange("(n p) o -> p n o", p=128))
                yield
                # Q/K projections: psum[o128, t512] over 8 d-tiles
                for xs, w_sb, dst in ((xq_sb, wq_sb, QT), (xk_sb, wk_sb, KT)):
                    for u in range(2):
                        ps = ps_mm.tile([128, TQ], F32, name="ps")
                        for kd in range(8):
                            nc.tensor.matmul(
                                ps[:],
                                w_sb[:, kd, bass.ts(u, 128)],
                                xs[:, kd, :],
                                start=(kd == 0),
                                stop=(kd == 7),
                            )
                            yield
                        nc.vector.tensor_copy(dst[u][:, bass.ts(tcc, TQ)], ps[:])
                        yield
                # V projection: natural orientation [t128, feat256] per j-tile
                for jl in range(4):
                    jt = tcc * 4 + jl
                    psv = ps_mm.tile([128, TQ], F32, name="ps")
                    for kd in range(8):
                        nc.tensor.matmul(
                            psv[:, 0:HF],
                            xv_sb[:, kd, bass.ts(jl, 128)],
                            wv_sb[:, kd, :],
                            start=(kd == 0),
                            stop=(kd == 7),
                        )
                        if kd % 2 == 1:
                            yield
                    nc.vector.tensor_copy(
                        Vb[:, jt * HPC:(jt + 1) * HPC, 0:DH],
                        psv[:, 0:HF].rearrange("p (h k) -> p h k", k=DH),
                    )
                    yield

            def outproj_units(t):
                """Generator: output projection for chunk t, split across the
                4-core group by position strip: rank r computes rows
                [128r, 128r+128) of chunk t via a register-offset DMA on the
                gathered ctx. out_q row block t holds this core's strip."""
                cstt = cstp.tile([128, 8, 128], BF16, name="cst")
                ag_ap = agout[t][:, :].rearrange("(n p) t -> p n t", p=128)
                nc.sync.dma_start(cstt[:], ag_ap[:, :, bass.ts(rank, 128)])
                yield
                for dc in range(2):
                    pso = ps_mm.tile([128, TQ], F32, name="ps")
                    for kt in range(8):
                        nc.tensor.matmul(
                            pso[:],
                            cstt[:, kt, :],
                            wo_sb[:, kt, bass.ts(dc, TQ)],
                            start=(kt == 0),
                            stop=(kt == 7),
                        )
                        yield
                    ob = obp.tile([128, TQ], F32, name="ob")
                    nc.vector.tensor_copy(ob[:], pso[:])
                    nc.sync.dma_start(
                        out_q[bass.ts(t, 128), bass.ts(dc, TQ)], ob[:]
                    )
                    yield

            def pull(gens, n):
                """Pull up to n work units from the generator queue."""
                while gens and n > 0:
                    if next(gens[0], "END") == "END":
                        gens.pop(0)
                    else:
                        n -= 1

            for rep in range(reps):
                # Chunk 0's x-DMAs + projections run un-interleaved (there
                # is nothing before them to hide behind).
                gens = [proj_units(0, first=(rep == 0))]
                pull(gens, 10 ** 6)
                carry = None  # previous chunk's outproj generator

                for ic in range(4):
                    if rep == 0 and ic == 0:
                        nc.sync.dma_start(
                            wo_sb[:],
                            woT[:, :].rearrange("(n p) d -> p n d", p=128))
                    # Interleave into this chunk's attention slots: the NEXT
                    # chunk's x-DMAs + projections (must finish before that
                    # chunk's attention), then the PREVIOUS chunk's output
                    # projection (flexible; its cst DMA waits on a collective
                    # so it must not be emitted at the head of the chunk or
                    # it head-of-line-blocks the SP DMA queue).
                    if ic < 3:
                        gens.append(proj_units(ic + 1, first=False))
                    if carry is not None:
                        gens.append(carry)
                        carry = None
                    n_jt = 4 * ic + 4
                    for h in range(HPC):
                        u, po = h // 2, (h % 2) * DH
                        pctx = ps_ctx.tile([DH + 1, TQ], F32, name="pctx")
                        ats = []
                        lows = []
                        for jt in range(n_jt):
                            p = jt - 4 * ic
                            lo = p * 128 if p > 0 else 0
                            ps = ps_s.tile([128, TQ], F32, name="ps_sc")
                            nc.tensor.matmul(
                                ps[:, lo:TQ],
                                KT[u][po:po + DH, bass.ts(jt, 128)],
                                QT[u][po:po + DH, ic * TQ + lo:(ic + 1) * TQ],
                                start=True,
                                stop=True,
                            )
                            at = apool.tile([128, TQ], BF16, name="at")
                            if p >= 0:
                                nc.vector.tensor_add(
                                    ps[:, bass.ts(p, 128)],
                                    ps[:, bass.ts(p, 128)],
                                    dmask_sb[:],
                                )
                            nc.scalar.activation(
                                at[:, lo:TQ], ps[:, lo:TQ], Exp, scale=SCALE
                            )
                            ats.append(at)
                            lows.append(lo)
                            # AV accumulation skewed 1 behind scores for
                            # PE/ACT pipelining; diagonal tiles only cover
                            # columns >= their lo (rest contribute nothing).
                            if jt >= 1:
                                pv = jt - 1
                                pl = lows[pv]
                                nc.tensor.matmul(
                                    pctx[:, pl:TQ],
                                    Vb[:, pv * HPC + h, :],
                                    ats[pv][:, pl:TQ],
                                    start=(pv == 0),
                                    stop=False,
                                    skip_group_check=True,
                                )
                            # Interleave next-chunk projection / prev-chunk
                            # output-projection work into this slot.
                            pull(gens, 2)
                        pv = n_jt - 1
                        pl = lows[pv]
                        nc.tensor.matmul(
                            pctx[:, pl:TQ],
                            Vb[:, pv * HPC + h, :],
                            ats[pv][:, pl:TQ],
                            start=(pv == 0),
                            stop=True,
                            skip_group_check=True,
                        )

                        # Normalize: row DH of pctx is the denominator.
                        # Broadcast 1/denom across 64 partitions via a rank-1
                        # bf16 matmul (1 cyc/row; bf16 rounding of 1/denom is
                        # ~0.2% rel, well within the gate).
                        rv = rvpool.tile([1, TQ], BF16, name="rvec")
                        with nc.allow_low_precision(reason="bf16 1/denom"):
                            nc.vector.reciprocal(rv[:], pctx[DH:DH + 1, :])
                        pb = ps_b.tile([DH, TQ], F32, name="pb")
                        nc.tensor.matmul(
                            pb[:], ones_sb[:], rv[:], start=True, stop=True,
                        )
                        rb = rbpool.tile([DH, TQ], F32, name="rbt")
                        nc.vector.tensor_copy(rb[:], pb[:])
                        ctxT = cpool.tile([DH, TQ], BF16, name="ctxT")
                        nc.vector.tensor_mul(ctxT[:], pctx[0:DH, :], rb[:])
                        nc.sync.dma_start(
                            ccin[ic][h * DH:(h + 1) * DH, :], ctxT[:]
                        )

                    # Force-drain leftovers: the next chunk's projections MUST
                    # be fully emitted before its attention begins (the PE
                    # queue is in-order), and lingering outproj finishes too.
                    pull(gens, 10 ** 6)

                    # Gather all 16 heads' ctxT for this i-chunk across the
                    # 4-core group (concat by group rank).
                    nc.gpsimd.collective_compute(
                        "AllGather",
                        mybir.AluOpType.bypass,
                        replica_groups=GROUPS,
                        ins=[ccin[ic].opt()],
                        outs=[agout[ic].opt()],
                    )
                    carry = outproj_units(ic)

                # Tail: the last chunk's output projection.
                gens = [carry]
                pull(gens, 10 ** 6)

    nc.finalize()
    return nc


def _make_in_maps(inputs):
    query, key, value = inputs["query"], inputs["key"], inputs["value"]
    mask = inputs["mask"]
    Wq, Wk, Wv, Wo = inputs["Wq"], inputs["Wk"], inputs["Wv"], inputs["Wo"]

    dmask_blk = np.where(
        np.asarray(mask[:128, :128]).T, np.float32(0.0), np.float32(-1e9)
    ).astype(np.float32)
    woT_full = np.ascontiguousarray(np.asarray(Wo, np.float32).T.astype(NPBF16))

    def bt(a):
        return np.ascontiguousarray(np.asarray(a, np.float32).T.astype(NPBF16))

    in_maps = []
    for c in range(8):
        b, r = divmod(c, 4)
        rs = slice(r * HF, (r + 1) * HF)
        in_maps.append(
            {
                "xqT": bt(query[b]),
                "xkT": bt(key[b]),
                "xvT": bt(value[b]),
                "wqT": bt(Wq[rs]),
                "wkT": bt(Wk[rs]),
                "wvT": bt(Wv[rs]),
                "woT": woT_full,
                "dmask": dmask_blk,
            }
        )
    return in_maps


def _run(inputs, trace=False):
    if "nc" not in _CACHE:
        _CACHE["nc"] = _build_graph()
    nc = _CACHE["nc"]
    in_maps = _make_in_maps(inputs)
    res = run_bass_kernel_spmd(nc, in_maps, core_ids=list(range(8)), trace=trace)

    out = np.empty((B, S, D), np.float32)
    for c in range(8):
        b, r = divmod(c, 4)
        oq = np.asarray(res.results[c]["out_q"])
        for t in range(4):
            lo = t * TQ + r * 128
            out[b, lo:lo + 128, :] = oq[t * 128:(t + 1) * 128, :]
    return out, res


def kernel(**inputs):
    out, _ = _run(inputs, trace=False)
    return out
